# revision 16
# baseline (speedup 1.0000x reference)
"""Trainium2 Bass kernel for nn_BaselineGPT (sliding-window GQA attention block).

Sharding: 8 cores = 2 batches x 4 sequence chunks of 512 queries.
Each core computes its 512 output rows end-to-end (QKV proj, RMS norm, RoPE,
windowed GQA attention, output proj).  KV halo of 256 rows comes with the
chunk; chunk 0's missing halo is masked via a -30000 bias folded into the
exp() activation's per-partition bias slot.  K-side rmsnorm is folded into
the exp() scale slot (rope commutes with per-head scaling).  Pair-head
mixing is folded into Wo on the host.

Perf notes: the PE p-state ramp means the tensor engine runs 2x faster when
continuously busy, so instruction order keeps the tensor queue dense; DMA
loads are split across the sync/scalar/gpsimd queues in dependency order so
the first matmul can start ~10us in.
"""

import math
from contextlib import ExitStack

import numpy as np

import concourse.bass as bass
from concourse import bacc
import concourse.mybir as mybir
import concourse.tile as tile
from concourse.masks import make_identity

B, S, DIM = 2, 2048, 1024
H, KVH, HD = 16, 4, 64
WINDOW = 256
ROPE_BASE = 10000.0
EPS = 1e-6

NQ = 512          # queries per core
NK = 768          # kv rows per core (incl 256 halo)
NCORES = 8
F32 = mybir.dt.float32
BF16 = mybir.dt.bfloat16

_BUILT = None


def _build():
    nc = bacc.Bacc(None)

    xt = nc.declare_dram_parameter("xt", [DIM, NK], BF16, isOutput=False)
    wq = nc.declare_dram_parameter("wq", [DIM, DIM], BF16, isOutput=False)
    wkv = nc.declare_dram_parameter("wkv", [DIM, 512], BF16, isOutput=False)
    wo = nc.declare_dram_parameter("wo", [DIM, DIM], BF16, isOutput=False)
    cs = nc.declare_dram_parameter("cs", [128, 6 * HD], F32, isOutput=False)
    kb = nc.declare_dram_parameter("kb", [128, 6], F32, isOutput=False)
    qg8 = nc.declare_dram_parameter("qg8", [128, H], F32, isOutput=False)
    m0 = nc.declare_dram_parameter("m0", [128, 512], BF16, isOutput=False)
    m2 = nc.declare_dram_parameter("m2", [128, 512], BF16, isOutput=False)
    out = nc.declare_dram_parameter("out", [NQ, DIM], BF16, isOutput=True)

    with tile.TileContext(nc) as tc, ExitStack() as ctx:
        const = ctx.enter_context(tc.tile_pool(name="const", bufs=1))
        big = ctx.enter_context(tc.tile_pool(name="big", bufs=1))
        tmp = ctx.enter_context(tc.tile_pool(name="tmp", bufs=3))
        att_pool = ctx.enter_context(tc.tile_pool(name="att", bufs=3))
        ob_pool = ctx.enter_context(tc.tile_pool(name="ob", bufs=2))
        ps_proj = ctx.enter_context(tc.tile_pool(name="psp", bufs=3, space="PSUM"))
        ps_pss = ctx.enter_context(tc.tile_pool(name="pss", bufs=3, space="PSUM"))
        ps_y = ctx.enter_context(tc.tile_pool(name="psy", bufs=2, space="PSUM"))

        # ---- preload DMAs, ordered by first use across the 3 dma queues ----
        xt_sb = [None] * 8
        wkv_sb = [None] * 8
        wq_sb = [None] * 8
        wo_sb = [None] * 8
        for kt_ in range(8):
            xt_sb[kt_] = big.tile([128, NK], BF16, tag=f"xt{kt_}", name=f"xt{kt_}")
            wkv_sb[kt_] = big.tile([128, 512], BF16, tag=f"wkv{kt_}", name=f"wkv{kt_}")
            wq_sb[kt_] = big.tile([128, DIM], BF16, tag=f"wq{kt_}", name=f"wq{kt_}")
            wo_sb[kt_] = big.tile([128, DIM], BF16, tag=f"wo{kt_}", name=f"wo{kt_}")
        cs_sb = const.tile([128, 6, HD], F32, tag="cs")
        kb_sb = const.tile([128, 6], F32, tag="kb")
        qg_sb = const.tile([128, H], F32, tag="qg")
        m02_sb = const.tile([128, 2, 512], BF16, tag="m02")

        def dram_tile(t, dram, kt_):
            return (t, dram[kt_ * 128: kt_ * 128 + 128, :])

        # interleave so the kt=0..7 accumulation chain unblocks in order
        sync_q = [dram_tile(wkv_sb[0], wkv, 0), dram_tile(xt_sb[0], xt, 0),
                  dram_tile(xt_sb[1], xt, 1), dram_tile(xt_sb[2], xt, 2),
                  dram_tile(wkv_sb[1], wkv, 1), dram_tile(wkv_sb[2], wkv, 2),
                  dram_tile(wq_sb[0], wq, 0), dram_tile(wq_sb[1], wq, 1),
                  dram_tile(wq_sb[2], wq, 2), dram_tile(wq_sb[3], wq, 3),
                  (m02_sb[:, 0, :], m0[:, :]),
                  dram_tile(wo_sb[0], wo, 0), dram_tile(wo_sb[1], wo, 1),
                  dram_tile(wo_sb[2], wo, 2), dram_tile(wo_sb[3], wo, 3)]
        scal_q = [dram_tile(xt_sb[3], xt, 3), dram_tile(xt_sb[4], xt, 4),
                  dram_tile(xt_sb[5], xt, 5), dram_tile(wkv_sb[3], wkv, 3),
                  dram_tile(wkv_sb[4], wkv, 4),
                  dram_tile(wq_sb[4], wq, 4), dram_tile(wq_sb[5], wq, 5),
                  dram_tile(wq_sb[6], wq, 6), dram_tile(wq_sb[7], wq, 7),
                  (m02_sb[:, 1, :], m2[:, :]),
                  dram_tile(wo_sb[4], wo, 4), dram_tile(wo_sb[5], wo, 5),
                  dram_tile(wo_sb[6], wo, 6), dram_tile(wo_sb[7], wo, 7)]
        gps_q = [dram_tile(xt_sb[6], xt, 6), dram_tile(xt_sb[7], xt, 7),
                 (cs_sb.rearrange("p a b -> p (a b)"), cs[:, :]),
                 dram_tile(wkv_sb[5], wkv, 5), dram_tile(wkv_sb[6], wkv, 6),
                 dram_tile(wkv_sb[7], wkv, 7),
                 (kb_sb, kb[:, :]), (qg_sb, qg8[:, :])]
        for t, src in sync_q:
            nc.sync.dma_start(out=t, in_=src)
        for t, src in scal_q[:5]:
            nc.scalar.dma_start(out=t, in_=src)
        # preload activation tables (Exp/Sqrt/Copy) before first real use
        warm = const.tile([128, 1], F32, tag="warm")
        nc.vector.memset(warm, 1.0)
        nc.scalar.activation(out=warm, in_=warm,
                             func=mybir.ActivationFunctionType.Exp)
        nc.scalar.activation(out=warm, in_=warm,
                             func=mybir.ActivationFunctionType.Sqrt)
        nc.scalar.copy(out=warm, in_=warm)
        for t, src in scal_q[5:]:
            nc.scalar.dma_start(out=t, in_=src)
        for t, src in gps_q:
            nc.gpsimd.dma_start(out=t, in_=src)

        # ---- constants ----
        ident = const.tile([128, 128], BF16, tag="ident")
        make_identity(nc, ident)
        eps_t = const.tile([128, 1], F32, tag="eps")
        nc.vector.memset(eps_t, EPS)
        ones64 = const.tile([1, 64], BF16, tag="ones64")
        nc.vector.memset(ones64, 1.0)

        # ---- persistent SBUF tensors ----
        k_rope = big.tile([128, 6, KVH * HD], BF16, tag="krope")
        q_rope = big.tile([128, 4, DIM], BF16, tag="qrope")
        v_sb = big.tile([128, 6, KVH, HD + 1], BF16, tag="v")
        kt_sb = big.tile([64, KVH, NK], BF16, tag="kt")
        qt_sb = big.tile([64, 16, 512], BF16, tag="qt")
        yt_sb = big.tile([128, 8, 512], BF16, tag="yt")
        invk = big.tile([128, 6, KVH], F32, tag="invk")
        nc.vector.memset(v_sb[:, :, :, HD:HD + 1], 1.0)

        def rope(eng, dst, src, nh, st, tmp_tag):
            """dst[:, h, 0:32] = r1*cos + r2*sin ; dst[:, h, 32:64] = r2*cos - r1*sin"""
            hd2 = HD // 2
            r1 = src[:, :, 0:hd2]
            r2 = src[:, :, hd2:HD]
            cosb = cs_sb[:, st, 0:hd2].rearrange("p (o f) -> p o f", o=1).broadcast_to(
                (128, nh, hd2))
            sinb = cs_sb[:, st, hd2:HD].rearrange("p (o f) -> p o f", o=1).broadcast_to(
                (128, nh, hd2))
            t1 = tmp.tile([128, nh, hd2], BF16, tag=tmp_tag)
            t2 = tmp.tile([128, nh, hd2], BF16, tag=tmp_tag + "b")
            eng.tensor_mul(out=t1, in0=r1, in1=cosb)
            eng.tensor_mul(out=t2, in0=r2, in1=sinb)
            eng.tensor_add(out=dst[:, :, 0:hd2], in0=t1, in1=t2)
            eng.tensor_mul(out=t1, in0=r2, in1=cosb)
            eng.tensor_mul(out=t2, in0=r1, in1=sinb)
            eng.tensor_sub(out=dst[:, :, hd2:HD], in0=t1, in1=t2)

        # ---- fused KV (+Q) projection over the 6 kv s-tiles ----
        for st in range(6):
            pkv = ps_proj.tile([128, 512], F32, tag="pp")
            for kt_ in range(8):
                nc.tensor.matmul(
                    out=pkv,
                    lhsT=xt_sb[kt_][:, st * 128: st * 128 + 128],
                    rhs=wkv_sb[kt_],
                    start=(kt_ == 0),
                    stop=(kt_ == 7),
                )
            # K: rope (raw; norm folded into exp scale), V: copy, sumsq of k_rope
            kraw = tmp.tile([128, KVH, HD], BF16, tag="kraw")
            nc.scalar.copy(
                out=kraw, in_=pkv[:, 0:KVH * HD].rearrange("p (g d) -> p g d", d=HD))
            kr = k_rope[:, st, :].rearrange("p (g d) -> p g d", d=HD)
            rope(nc.gpsimd, kr, kraw, KVH, st, "kr")
            nc.scalar.copy(
                out=v_sb[:, st, :, 0:HD],
                in_=pkv[:, KVH * HD:].rearrange("p (g d) -> p g d", d=HD),
            )
            sqk = tmp.tile([128, KVH, HD], F32, tag="sqk")
            nc.gpsimd.tensor_mul(out=sqk, in0=kr, in1=kr)
            ssqk = tmp.tile([128, KVH], F32, tag="ssqk")
            nc.vector.tensor_reduce(
                out=ssqk, in_=sqk,
                axis=mybir.AxisListType.X, op=mybir.AluOpType.add)
            nc.scalar.activation(
                out=ssqk, in_=ssqk, func=mybir.ActivationFunctionType.Sqrt,
                bias=eps_t, scale=1.0 / HD)
            nc.vector.reciprocal(out=invk[:, st, :], in_=ssqk)
            # K transpose: k_rope [128s, (g d)] -> kt_sb [d, g, 128s-block]
            ptk = ps_pss.tile([128, 512], BF16, tag="ps")
            for g in range(KVH):
                nc.tensor.transpose(
                    out=ptk[0:HD, g * 128: g * 128 + 128],
                    in_=k_rope[:, st, g * HD: g * HD + HD],
                    identity=ident,
                )
            nc.vector.tensor_copy(
                out=kt_sb[:, :, st * 128: st * 128 + 128],
                in_=ptk[0:HD, :].rearrange("p (g s) -> p g s", s=128),
            )

            if st >= 2:
                qst = st - 2
                qraw = tmp.tile([128, H, HD], BF16, tag="qraw")
                for half in range(2):
                    pq = ps_proj.tile([128, 512], F32, tag="pp")
                    for kt_ in range(8):
                        nc.tensor.matmul(
                            out=pq,
                            lhsT=xt_sb[kt_][:, st * 128: st * 128 + 128],
                            rhs=wq_sb[kt_][:, half * 512: half * 512 + 512],
                            start=(kt_ == 0),
                            stop=(kt_ == 7),
                        )
                    nc.scalar.copy(
                        out=qraw[:, half * 8: half * 8 + 8, :],
                        in_=pq.rearrange("p (h d) -> p h d", d=HD),
                    )
                qr = q_rope[:, qst, :].rearrange("p (h d) -> p h d", d=HD)
                rope(nc.vector, qr, qraw, H, st, "qr")
                # rms norm: sumsq on post-rope q (rotation preserves norms)
                sqq = tmp.tile([128, H, HD], BF16, tag="sqq")
                ssqq = tmp.tile([128, H], F32, tag="ssqq")
                nc.gpsimd.tensor_mul(out=sqq, in0=qr, in1=qr)
                nc.vector.tensor_reduce(
                    out=ssqq, in_=sqq, axis=mybir.AxisListType.X,
                    op=mybir.AluOpType.add)
                nc.scalar.activation(
                    out=ssqq, in_=ssqq, func=mybir.ActivationFunctionType.Sqrt,
                    bias=eps_t, scale=1.0 / HD)
                invq = tmp.tile([128, H], F32, tag="invq")
                nc.vector.reciprocal(out=invq, in_=ssqq)
                nc.vector.tensor_mul(out=invq, in0=invq, in1=qg_sb)
                nc.vector.tensor_mul(
                    out=qr, in0=qr,
                    in1=invq.rearrange("p (h o) -> p h o", o=1).broadcast_to(
                        (128, H, HD)))
                # Q transpose per group -> qt_sb[:, g*4+qst, :]
                for g in range(KVH):
                    ptq = ps_pss.tile([128, 512], BF16, tag="ps")
                    for hh in range(4):
                        h = g * 4 + hh
                        nc.tensor.transpose(
                            out=ptq[0:HD, hh * 128: hh * 128 + 128],
                            in_=q_rope[:, qst, h * HD: h * HD + HD],
                            identity=ident,
                        )
                    nc.scalar.copy(
                        out=qt_sb[:, g * 4 + qst, :], in_=ptq[0:HD, :])

        # ---- attention, qb-major with software pipelining ----
        # att slots: t=0 -> 0, t=2 -> 1 (mask-adjacent), t=1 -> 2 (unmasked)
        SLOT = {0: 0, 2: 1, 1: 2}

        def emit_scores(qb, g):
            att = att_pool.tile([128, 3, 512], BF16, tag="att")
            for t in (0, 2, 1):
                pss = ps_pss.tile([128, 512], F32, tag="ps")
                nc.tensor.matmul(
                    out=pss,
                    lhsT=kt_sb[:, g, qb * 128 + t * 128: qb * 128 + t * 128 + 128],
                    rhs=qt_sb[:, g * 4 + qb, :],
                    start=True, stop=True,
                )
                nc.scalar.activation(
                    out=att[:, SLOT[t], :], in_=pss,
                    func=mybir.ActivationFunctionType.Exp,
                    bias=kb_sb[:, qb + t: qb + t + 1],
                    scale=invk[:, qb + t, g: g + 1],
                )
                if t == 2:
                    nc.gpsimd.tensor_mul(
                        out=att[:, 0:2, :], in0=att[:, 0:2, :], in1=m02_sb)
            return att

        def emit_attv(qb, g, att):
            psy = ps_y.tile([128, 512], F32, tag="py")
            for t in (1, 0, 2):
                nc.tensor.matmul(
                    out=psy[0:HD + 1, :],
                    lhsT=v_sb[:, qb + t, g, :],
                    rhs=att[:, SLOT[t], :],
                    start=(t == 1), stop=(t == 2),
                )
            rec = tmp.tile([1, 512], F32, tag="rec")
            nc.vector.reciprocal(out=rec, in_=psy[64:65, :])
            rec_bf = tmp.tile([1, 512], BF16, tag="recb")
            nc.vector.tensor_copy(out=rec_bf, in_=rec)
            return psy, rec_bf

        def emit_norm(qb, g, psy, rec):
            prb = ps_pss.tile([128, 512], F32, tag="ps")
            nc.tensor.matmul(
                out=prb[0:HD, :], lhsT=ones64,
                rhs=rec, start=True, stop=True)
            rbb = tmp.tile([HD, 512], BF16, tag="rbb")
            nc.vector.tensor_copy(out=rbb, in_=prb[0:HD, :])
            psy4 = psy[0:HD, :].rearrange("p (h s) -> p h s", s=128)
            prb4 = rbb.rearrange("p (h s) -> p h s", s=128)
            for lo in range(2):
                nc.vector.tensor_mul(
                    out=yt_sb[lo * 64: lo * 64 + 64, 2 * g: 2 * g + 2,
                              qb * 128: qb * 128 + 128],
                    in0=psy4[:, 2 * lo: 2 * lo + 2, :],
                    in1=prb4[:, 2 * lo: 2 * lo + 2, :],
                )

        def emit_outproj(qb):
            ob = ob_pool.tile([128, DIM], BF16, tag="ob")
            for half in range(2):
                po = ps_pss.tile([128, 512], F32, tag="ps")
                for p in range(8):
                    nc.tensor.matmul(
                        out=po,
                        lhsT=yt_sb[:, p, qb * 128: qb * 128 + 128],
                        rhs=wo_sb[p][:, half * 512: half * 512 + 512],
                        start=(p == 0), stop=(p == 7),
                    )
                nc.vector.tensor_copy(
                    out=ob[:, half * 512: half * 512 + 512], in_=po)
            nc.sync.dma_start(out=out[qb * 128: qb * 128 + 128, :], in_=ob)

        for qb in range(4):
            atts, psys, recs = {}, {}, {}
            atts[0] = emit_scores(qb, 0)
            atts[1] = emit_scores(qb, 1)
            for g in range(KVH):
                psys[g], recs[g] = emit_attv(qb, g, atts[g])
                if g + 2 < KVH:
                    atts[g + 2] = emit_scores(qb, g + 2)
                if g >= 1:
                    emit_norm(qb, g - 1, psys[g - 1], recs[g - 1])
            emit_norm(qb, 3, psys[3], recs[3])
            if qb >= 1:
                emit_outproj(qb - 1)
        emit_outproj(3)

    nc.finalize()
    return nc


def _host_inputs(x, Wq, Wk, Wv, Wo, q_gain, pair_mix):
    """Build the 8 per-core input maps."""
    x = np.asarray(x, np.float32)
    Wq = np.asarray(Wq, np.float32)
    Wk = np.asarray(Wk, np.float32)
    Wv = np.asarray(Wv, np.float32)
    Wo = np.asarray(Wo, np.float32)
    q_gain = np.asarray(q_gain, np.float32)
    pair_mix = np.asarray(pair_mix, np.float32)

    # fold pair mixing into Wo:  out = y_mix @ Wo.T,  y_mix = y @ M.T  =>  Wo' = Wo @ M
    M = np.zeros((DIM, DIM), np.float32)
    eye = np.eye(HD, dtype=np.float32)
    for p in range(H // 2):
        for o in range(2):
            for i in range(2):
                ho, hi = 2 * p + o, 2 * p + i
                M[ho * HD: ho * HD + HD, hi * HD: hi * HD + HD] = (
                    pair_mix[p, o, i] * eye
                )
    woT = np.ascontiguousarray((Wo @ M).T)  # [in=(h,d), out]
    # permute rows into the yt pair layout: row blk*128 + lo*64 + d
    # holds head h = 4*(blk//2) + 2*lo + blk%2, dim d
    perm = np.empty(DIM, np.int64)
    for blk in range(8):
        for lo in range(2):
            h = 4 * (blk // 2) + 2 * lo + (blk % 2)
            perm[blk * 128 + lo * 64: blk * 128 + lo * 64 + HD] = (
                np.arange(HD) + h * HD)
    woT = woT[perm]

    wqT = np.ascontiguousarray(Wq.T)
    wkvT = np.ascontiguousarray(np.concatenate([Wk.T, Wv.T], axis=1))
    qg8 = np.tile((q_gain / math.sqrt(HD)).reshape(1, H), (128, 1)).astype(np.float32)

    inv_freq = 1.0 / (ROPE_BASE ** (np.arange(0, HD, 2, dtype=np.float32) / HD))

    ql = np.arange(128)
    m0_ = (ql[:, None] >= ql[None, :] + 1).astype(np.float32)  # kl >= ql+1
    m2_ = (ql[:, None] <= ql[None, :]).astype(np.float32)      # kl <= ql
    m0t = np.ascontiguousarray(np.tile(m0_, (1, 4)))
    m2t = np.ascontiguousarray(np.tile(m2_, (1, 4)))

    import ml_dtypes
    bf = ml_dtypes.bfloat16
    wqT, wkvT, woT = (a.astype(bf) for a in (wqT, wkvT, woT))
    m0t, m2t = m0t.astype(bf), m2t.astype(bf)
    sel_np = np.zeros((4, 256), np.float32)
    for g in range(4):
        sel_np[g, g * 64:(g + 1) * 64] = 1.0
    sel_np = sel_np.astype(bf)
    in_maps = []
    for core in range(NCORES):
        b, c = core // 4, core % 4
        ks = 512 * c - 256
        xc = np.zeros((NK, DIM), np.float32)
        lo = max(0, ks)
        xc[lo - ks:] = x[b, lo: ks + NK]
        pos = ks + np.arange(NK, dtype=np.float32)
        freqs = pos[:, None] * inv_freq[None, :]        # [NK, 32]
        # cs[p, st*64 + j]: j<32 cos, j>=32 sin, for kv row st*128+p
        csk = np.concatenate([np.cos(freqs), np.sin(freqs)], axis=1)  # [NK, 64]
        csk = np.ascontiguousarray(
            csk.reshape(6, 128, HD).transpose(1, 0, 2).reshape(128, 6 * HD))
        kbias = np.where(pos < 0, -30000.0, 0.0).astype(np.float32)
        kbias = np.ascontiguousarray(kbias.reshape(6, 128).T)  # [128, 6]
        in_maps.append(
            {
                "xt": np.ascontiguousarray(xc.T).astype(bf),
                "wq": wqT,
                "wkv": wkvT,
                "wo": woT,
                "cs": csk,
                "kb": kbias,
                "qg8": qg8,
                "m0": m0t,
                "m2": m2t,
                "sel": sel_np,
            }
        )
    return in_maps


def kernel(x, Wq, Wk, Wv, Wo, q_gain, pair_mix):
    global _BUILT
    from concourse.bass_utils import run_bass_kernel_spmd

    if _BUILT is None:
        _BUILT = _build()
    in_maps = _host_inputs(x, Wq, Wk, Wv, Wo, q_gain, pair_mix)
    res = run_bass_kernel_spmd(_BUILT, in_maps, list(range(NCORES)))
    out = np.empty((B, S, DIM), np.float32)
    for core in range(NCORES):
        b, c = core // 4, core % 4
        out[b, 512 * c: 512 * c + 512, :] = res.results[core]["out"].astype(np.float32)
    return out


# revision 18
# speedup vs baseline: 1.0540x; 1.0540x over previous
"""Trainium2 Bass kernel for nn_BaselineGPT (sliding-window GQA attention block).

Sharding: 8 cores = 2 batches x 4 sequence chunks of 512 queries.
Each core computes its 512 output rows end-to-end (QKV proj, RMS norm, RoPE,
windowed GQA attention, output proj).  KV halo of 256 rows comes with the
chunk; chunk 0's missing halo is masked via a -30000 bias folded into the
exp() activation's per-partition bias slot.  K-side rmsnorm is folded into
the exp() scale slot (rope commutes with per-head scaling).  Pair-head
mixing is folded into Wo on the host.

Perf notes: the PE p-state ramp means the tensor engine runs 2x faster when
continuously busy, so instruction order keeps the tensor queue dense; DMA
loads are split across the sync/scalar/gpsimd queues in dependency order so
the first matmul can start ~10us in.
"""

import math
from contextlib import ExitStack

import numpy as np

import concourse.bass as bass
from concourse import bacc
import concourse.mybir as mybir
import concourse.tile as tile
from concourse.masks import make_identity

B, S, DIM = 2, 2048, 1024
H, KVH, HD = 16, 4, 64
WINDOW = 256
ROPE_BASE = 10000.0
EPS = 1e-6

NQ = 512          # queries per core
NK = 768          # kv rows per core (incl 256 halo)
NCORES = 8
F32 = mybir.dt.float32
BF16 = mybir.dt.bfloat16

_BUILT = None


def _build():
    nc = bacc.Bacc(None)

    xt = nc.declare_dram_parameter("xt", [DIM, NK], BF16, isOutput=False)
    wq = nc.declare_dram_parameter("wq", [DIM, DIM], BF16, isOutput=False)
    wkv = nc.declare_dram_parameter("wkv", [DIM, 512], BF16, isOutput=False)
    wo = nc.declare_dram_parameter("wo", [DIM, DIM], BF16, isOutput=False)
    cs = nc.declare_dram_parameter("cs", [128, 6 * HD], F32, isOutput=False)
    kb = nc.declare_dram_parameter("kb", [128, 6], F32, isOutput=False)
    qg8 = nc.declare_dram_parameter("qg8", [128, H], F32, isOutput=False)
    m0 = nc.declare_dram_parameter("m0", [128, 512], BF16, isOutput=False)
    m2 = nc.declare_dram_parameter("m2", [128, 512], BF16, isOutput=False)
    out = nc.declare_dram_parameter("out", [NQ, DIM], BF16, isOutput=True)

    with tile.TileContext(nc) as tc, ExitStack() as ctx:
        const = ctx.enter_context(tc.tile_pool(name="const", bufs=1))
        big = ctx.enter_context(tc.tile_pool(name="big", bufs=1))
        tmp = ctx.enter_context(tc.tile_pool(name="tmp", bufs=3))
        att_pool = ctx.enter_context(tc.tile_pool(name="att", bufs=5))
        ob_pool = ctx.enter_context(tc.tile_pool(name="ob", bufs=2))
        ps_proj = ctx.enter_context(tc.tile_pool(name="psp", bufs=3, space="PSUM"))
        ps_pss = ctx.enter_context(tc.tile_pool(name="pss", bufs=3, space="PSUM"))
        ps_y = ctx.enter_context(tc.tile_pool(name="psy", bufs=2, space="PSUM"))

        # ---- preload DMAs, ordered by first use across the 3 dma queues ----
        xt_sb = [None] * 8
        wkv_sb = [None] * 8
        wq_sb = [None] * 8
        wo_sb = [None] * 8
        for kt_ in range(8):
            xt_sb[kt_] = big.tile([128, NK], BF16, tag=f"xt{kt_}", name=f"xt{kt_}")
            wkv_sb[kt_] = big.tile([128, 512], BF16, tag=f"wkv{kt_}", name=f"wkv{kt_}")
            wq_sb[kt_] = big.tile([128, DIM], BF16, tag=f"wq{kt_}", name=f"wq{kt_}")
            wo_sb[kt_] = big.tile([128, DIM], BF16, tag=f"wo{kt_}", name=f"wo{kt_}")
        cs_sb = const.tile([128, 6, HD], F32, tag="cs")
        kb_sb = const.tile([128, 6], F32, tag="kb")
        qg_sb = const.tile([128, H], F32, tag="qg")
        m02_sb = const.tile([128, 2, 512], BF16, tag="m02")

        def dram_tile(t, dram, kt_):
            return (t, dram[kt_ * 128: kt_ * 128 + 128, :])

        # interleave so the kt=0..7 accumulation chain unblocks in order
        sync_q = [dram_tile(wkv_sb[0], wkv, 0), dram_tile(xt_sb[0], xt, 0),
                  dram_tile(xt_sb[1], xt, 1), dram_tile(xt_sb[2], xt, 2),
                  dram_tile(wkv_sb[1], wkv, 1), dram_tile(wkv_sb[2], wkv, 2),
                  dram_tile(wq_sb[0], wq, 0), dram_tile(wq_sb[1], wq, 1),
                  dram_tile(wq_sb[2], wq, 2), dram_tile(wq_sb[3], wq, 3),
                  (m02_sb[:, 0, :], m0[:, :]),
                  dram_tile(wo_sb[0], wo, 0), dram_tile(wo_sb[1], wo, 1),
                  dram_tile(wo_sb[2], wo, 2), dram_tile(wo_sb[3], wo, 3)]
        scal_q = [dram_tile(xt_sb[3], xt, 3), dram_tile(xt_sb[4], xt, 4),
                  dram_tile(xt_sb[5], xt, 5), dram_tile(wkv_sb[3], wkv, 3),
                  dram_tile(wkv_sb[4], wkv, 4),
                  dram_tile(wq_sb[4], wq, 4), dram_tile(wq_sb[5], wq, 5),
                  dram_tile(wq_sb[6], wq, 6), dram_tile(wq_sb[7], wq, 7),
                  (m02_sb[:, 1, :], m2[:, :]),
                  dram_tile(wo_sb[4], wo, 4), dram_tile(wo_sb[5], wo, 5),
                  dram_tile(wo_sb[6], wo, 6), dram_tile(wo_sb[7], wo, 7)]
        gps_q = [dram_tile(xt_sb[6], xt, 6), dram_tile(xt_sb[7], xt, 7),
                 (cs_sb.rearrange("p a b -> p (a b)"), cs[:, :]),
                 dram_tile(wkv_sb[5], wkv, 5), dram_tile(wkv_sb[6], wkv, 6),
                 dram_tile(wkv_sb[7], wkv, 7),
                 (kb_sb, kb[:, :]), (qg_sb, qg8[:, :])]
        for t, src in sync_q:
            nc.sync.dma_start(out=t, in_=src)
        for t, src in scal_q[:5]:
            nc.scalar.dma_start(out=t, in_=src)
        # preload activation tables (Exp/Sqrt/Copy) before first real use
        warm = const.tile([128, 1], F32, tag="warm")
        nc.vector.memset(warm, 1.0)
        nc.scalar.activation(out=warm, in_=warm,
                             func=mybir.ActivationFunctionType.Exp)
        nc.scalar.activation(out=warm, in_=warm,
                             func=mybir.ActivationFunctionType.Sqrt)
        nc.scalar.copy(out=warm, in_=warm)
        for t, src in scal_q[5:]:
            nc.scalar.dma_start(out=t, in_=src)
        for t, src in gps_q:
            nc.gpsimd.dma_start(out=t, in_=src)

        # ---- constants ----
        ident = const.tile([128, 128], BF16, tag="ident")
        make_identity(nc, ident)
        eps_t = const.tile([128, 1], F32, tag="eps")
        nc.vector.memset(eps_t, EPS)
        ones64 = const.tile([1, 64], BF16, tag="ones64")
        nc.vector.memset(ones64, 1.0)

        # ---- persistent SBUF tensors ----
        k_rope = big.tile([128, 6, KVH * HD], BF16, tag="krope")
        q_rope = big.tile([128, 4, DIM], BF16, tag="qrope")
        v_sb = big.tile([128, 6, KVH, HD + 1], BF16, tag="v")
        kt_sb = big.tile([64, KVH, NK], BF16, tag="kt")
        qt_sb = big.tile([64, 16, 512], BF16, tag="qt")
        yt_sb = big.tile([128, 8, 512], BF16, tag="yt")
        invk = big.tile([128, 6, KVH], F32, tag="invk")
        nc.vector.memset(v_sb[:, :, :, HD:HD + 1], 1.0)

        def rope(eng, dst, src, nh, st, tmp_tag):
            """dst[:, h, 0:32] = r1*cos + r2*sin ; dst[:, h, 32:64] = r2*cos - r1*sin"""
            hd2 = HD // 2
            r1 = src[:, :, 0:hd2]
            r2 = src[:, :, hd2:HD]
            cosb = cs_sb[:, st, 0:hd2].rearrange("p (o f) -> p o f", o=1).broadcast_to(
                (128, nh, hd2))
            sinb = cs_sb[:, st, hd2:HD].rearrange("p (o f) -> p o f", o=1).broadcast_to(
                (128, nh, hd2))
            t1 = tmp.tile([128, nh, hd2], BF16, tag=tmp_tag)
            t2 = tmp.tile([128, nh, hd2], BF16, tag=tmp_tag + "b")
            eng.tensor_mul(out=t1, in0=r1, in1=cosb)
            eng.tensor_mul(out=t2, in0=r2, in1=sinb)
            eng.tensor_add(out=dst[:, :, 0:hd2], in0=t1, in1=t2)
            eng.tensor_mul(out=t1, in0=r2, in1=cosb)
            eng.tensor_mul(out=t2, in0=r1, in1=sinb)
            eng.tensor_sub(out=dst[:, :, hd2:HD], in0=t1, in1=t2)

        # ---- fused KV (+Q) projection, one kv s-tile ----
        def emit_stile(st):
            pkv = ps_proj.tile([128, 512], F32, tag="pp")
            for kt_ in range(8):
                nc.tensor.matmul(
                    out=pkv,
                    lhsT=xt_sb[kt_][:, st * 128: st * 128 + 128],
                    rhs=wkv_sb[kt_],
                    start=(kt_ == 0),
                    stop=(kt_ == 7),
                )
            # K: rope (raw; norm folded into exp scale), V: copy, sumsq of k_rope
            kraw = tmp.tile([128, KVH, HD], BF16, tag="kraw")
            nc.scalar.copy(
                out=kraw, in_=pkv[:, 0:KVH * HD].rearrange("p (g d) -> p g d", d=HD))
            kr = k_rope[:, st, :].rearrange("p (g d) -> p g d", d=HD)
            rope(nc.gpsimd, kr, kraw, KVH, st, "kr")
            nc.scalar.copy(
                out=v_sb[:, st, :, 0:HD],
                in_=pkv[:, KVH * HD:].rearrange("p (g d) -> p g d", d=HD),
            )
            sqk = tmp.tile([128, KVH, HD], F32, tag="sqk")
            nc.gpsimd.tensor_mul(out=sqk, in0=kr, in1=kr)
            ssqk = tmp.tile([128, KVH], F32, tag="ssqk")
            nc.vector.tensor_reduce(
                out=ssqk, in_=sqk,
                axis=mybir.AxisListType.X, op=mybir.AluOpType.add)
            nc.scalar.activation(
                out=ssqk, in_=ssqk, func=mybir.ActivationFunctionType.Sqrt,
                bias=eps_t, scale=1.0 / HD)
            nc.vector.reciprocal(out=invk[:, st, :], in_=ssqk)
            # K transpose: k_rope [128s, (g d)] -> kt_sb [d, g, 128s-block]
            ptk = ps_pss.tile([128, 512], BF16, tag="ps")
            for g in range(KVH):
                nc.tensor.transpose(
                    out=ptk[0:HD, g * 128: g * 128 + 128],
                    in_=k_rope[:, st, g * HD: g * HD + HD],
                    identity=ident,
                )
            nc.vector.tensor_copy(
                out=kt_sb[:, :, st * 128: st * 128 + 128],
                in_=ptk[0:HD, :].rearrange("p (g s) -> p g s", s=128),
            )

            if st >= 2:
                qst = st - 2
                qraw = tmp.tile([128, H, HD], BF16, tag="qraw")
                for half in range(2):
                    pq = ps_proj.tile([128, 512], F32, tag="pp")
                    for kt_ in range(8):
                        nc.tensor.matmul(
                            out=pq,
                            lhsT=xt_sb[kt_][:, st * 128: st * 128 + 128],
                            rhs=wq_sb[kt_][:, half * 512: half * 512 + 512],
                            start=(kt_ == 0),
                            stop=(kt_ == 7),
                        )
                    nc.scalar.copy(
                        out=qraw[:, half * 8: half * 8 + 8, :],
                        in_=pq.rearrange("p (h d) -> p h d", d=HD),
                    )
                qr = q_rope[:, qst, :].rearrange("p (h d) -> p h d", d=HD)
                rope(nc.vector, qr, qraw, H, st, "qr")
                # rms norm: sumsq on post-rope q (rotation preserves norms)
                sqq = tmp.tile([128, H, HD], BF16, tag="sqq")
                ssqq = tmp.tile([128, H], F32, tag="ssqq")
                nc.gpsimd.tensor_mul(out=sqq, in0=qr, in1=qr)
                nc.vector.tensor_reduce(
                    out=ssqq, in_=sqq, axis=mybir.AxisListType.X,
                    op=mybir.AluOpType.add)
                nc.scalar.activation(
                    out=ssqq, in_=ssqq, func=mybir.ActivationFunctionType.Sqrt,
                    bias=eps_t, scale=1.0 / HD)
                invq = tmp.tile([128, H], F32, tag="invq")
                nc.vector.reciprocal(out=invq, in_=ssqq)
                nc.vector.tensor_mul(out=invq, in0=invq, in1=qg_sb)
                nc.vector.tensor_mul(
                    out=qr, in0=qr,
                    in1=invq.rearrange("p (h o) -> p h o", o=1).broadcast_to(
                        (128, H, HD)))
                # Q transpose per group -> qt_sb[:, g*4+qst, :]
                for g in range(KVH):
                    ptq = ps_pss.tile([128, 512], BF16, tag="ps")
                    for hh in range(4):
                        h = g * 4 + hh
                        nc.tensor.transpose(
                            out=ptq[0:HD, hh * 128: hh * 128 + 128],
                            in_=q_rope[:, qst, h * HD: h * HD + HD],
                            identity=ident,
                        )
                    nc.scalar.copy(
                        out=qt_sb[:, g * 4 + qst, :], in_=ptq[0:HD, :])

        # ---- attention blocks, interleaved with proj stiles ----
        # att slots: t=0 -> 0, t=2 -> 1 (mask-adjacent), t=1 -> 2 (unmasked)
        SLOT = {0: 0, 2: 1, 1: 2}

        def emit_scores(qb, g):
            att = att_pool.tile([128, 3, 512], BF16, tag="att")
            for t in (0, 2, 1):
                pss = ps_pss.tile([128, 512], F32, tag="ps")
                nc.tensor.matmul(
                    out=pss,
                    lhsT=kt_sb[:, g, qb * 128 + t * 128: qb * 128 + t * 128 + 128],
                    rhs=qt_sb[:, g * 4 + qb, :],
                    start=True, stop=True,
                )
                nc.scalar.activation(
                    out=att[:, SLOT[t], :], in_=pss,
                    func=mybir.ActivationFunctionType.Exp,
                    bias=kb_sb[:, qb + t: qb + t + 1],
                    scale=invk[:, qb + t, g: g + 1],
                )
                if t == 2:
                    nc.vector.tensor_mul(
                        out=att[:, 0, :], in0=att[:, 0, :], in1=m02_sb[:, 0, :])
                    nc.gpsimd.tensor_mul(
                        out=att[:, 1, :], in0=att[:, 1, :], in1=m02_sb[:, 1, :])
            return att

        def emit_attv(qb, g, att):
            psy = ps_y.tile([128, 512], F32, tag="py")
            for t in (1, 0, 2):
                nc.tensor.matmul(
                    out=psy[0:HD + 1, :],
                    lhsT=v_sb[:, qb + t, g, :],
                    rhs=att[:, SLOT[t], :],
                    start=(t == 1), stop=(t == 2),
                )
            den_s = tmp.tile([1, 512], F32, tag="dens")
            nc.scalar.copy(out=den_s, in_=psy[64:65, :])
            rec = tmp.tile([1, 512], F32, tag="rec")
            nc.vector.reciprocal_approx_fast(out=rec, in_=den_s)
            rec_bf = tmp.tile([1, 512], BF16, tag="recb")
            nc.gpsimd.tensor_copy(out=rec_bf, in_=rec)
            return psy, rec_bf

        def emit_norm(qb, g, psy, rec_bf):
            prb = ps_pss.tile([128, 512], F32, tag="ps")
            nc.tensor.matmul(
                out=prb[0:HD, :], lhsT=ones64,
                rhs=rec_bf, start=True, stop=True)
            rbb = tmp.tile([HD, 512], BF16, tag="rbb")
            nc.vector.tensor_copy(out=rbb, in_=prb[0:HD, :])
            psy4 = psy[0:HD, :].rearrange("p (h s) -> p h s", s=128)
            prb4 = rbb.rearrange("p (h s) -> p h s", s=128)
            for lo in range(2):
                nc.vector.tensor_mul(
                    out=yt_sb[lo * 64: lo * 64 + 64, 2 * g: 2 * g + 2,
                              qb * 128: qb * 128 + 128],
                    in0=psy4[:, 2 * lo: 2 * lo + 2, :],
                    in1=prb4[:, 2 * lo: 2 * lo + 2, :],
                )

        def emit_outproj(qb):
            ob = ob_pool.tile([128, DIM], BF16, tag="ob")
            for half in range(2):
                po = ps_pss.tile([128, 512], F32, tag="ps")
                for p in range(8):
                    nc.tensor.matmul(
                        out=po,
                        lhsT=yt_sb[:, p, qb * 128: qb * 128 + 128],
                        rhs=wo_sb[p][:, half * 512: half * 512 + 512],
                        start=(p == 0), stop=(p == 7),
                    )
                nc.vector.tensor_copy(
                    out=ob[:, half * 512: half * 512 + 512], in_=po)
            nc.sync.dma_start(out=out[qb * 128: qb * 128 + 128, :], in_=ob)

        def emit_scores_block(qb):
            return {g: emit_scores(qb, g) for g in range(KVH)}

        def emit_attv_block(qb, atts, outproj_qb=None):
            psys, recs = {}, {}
            for g in range(KVH):
                psys[g], recs[g] = emit_attv(qb, g, atts[g])
                if g >= 1:
                    emit_norm(qb, g - 1, psys[g - 1], recs[g - 1])
            emit_norm(qb, 3, psys[3], recs[3])
            if outproj_qb is not None:
                emit_outproj(outproj_qb)

        emit_stile(0)
        emit_stile(1)
        emit_stile(2)
        atts0 = emit_scores_block(0)
        emit_stile(3)
        emit_attv_block(0, atts0)
        atts1 = emit_scores_block(1)
        emit_stile(4)
        emit_attv_block(1, atts1, outproj_qb=0)
        atts2 = emit_scores_block(2)
        emit_stile(5)
        emit_attv_block(2, atts2, outproj_qb=1)
        atts3 = emit_scores_block(3)
        emit_attv_block(3, atts3, outproj_qb=2)
        emit_outproj(3)

    nc.finalize()
    return nc


def _host_inputs(x, Wq, Wk, Wv, Wo, q_gain, pair_mix):
    """Build the 8 per-core input maps."""
    x = np.asarray(x, np.float32)
    Wq = np.asarray(Wq, np.float32)
    Wk = np.asarray(Wk, np.float32)
    Wv = np.asarray(Wv, np.float32)
    Wo = np.asarray(Wo, np.float32)
    q_gain = np.asarray(q_gain, np.float32)
    pair_mix = np.asarray(pair_mix, np.float32)

    # fold pair mixing into Wo:  out = y_mix @ Wo.T,  y_mix = y @ M.T  =>  Wo' = Wo @ M
    M = np.zeros((DIM, DIM), np.float32)
    eye = np.eye(HD, dtype=np.float32)
    for p in range(H // 2):
        for o in range(2):
            for i in range(2):
                ho, hi = 2 * p + o, 2 * p + i
                M[ho * HD: ho * HD + HD, hi * HD: hi * HD + HD] = (
                    pair_mix[p, o, i] * eye
                )
    woT = np.ascontiguousarray((Wo @ M).T)  # [in=(h,d), out]
    # permute rows into the yt pair layout: row blk*128 + lo*64 + d
    # holds head h = 4*(blk//2) + 2*lo + blk%2, dim d
    perm = np.empty(DIM, np.int64)
    for blk in range(8):
        for lo in range(2):
            h = 4 * (blk // 2) + 2 * lo + (blk % 2)
            perm[blk * 128 + lo * 64: blk * 128 + lo * 64 + HD] = (
                np.arange(HD) + h * HD)
    woT = woT[perm]

    wqT = np.ascontiguousarray(Wq.T)
    wkvT = np.ascontiguousarray(np.concatenate([Wk.T, Wv.T], axis=1))
    qg8 = np.tile((q_gain / math.sqrt(HD)).reshape(1, H), (128, 1)).astype(np.float32)

    inv_freq = 1.0 / (ROPE_BASE ** (np.arange(0, HD, 2, dtype=np.float32) / HD))

    ql = np.arange(128)
    m0_ = (ql[:, None] >= ql[None, :] + 1).astype(np.float32)  # kl >= ql+1
    m2_ = (ql[:, None] <= ql[None, :]).astype(np.float32)      # kl <= ql
    m0t = np.ascontiguousarray(np.tile(m0_, (1, 4)))
    m2t = np.ascontiguousarray(np.tile(m2_, (1, 4)))

    import ml_dtypes
    bf = ml_dtypes.bfloat16
    wqT, wkvT, woT = (a.astype(bf) for a in (wqT, wkvT, woT))
    m0t, m2t = m0t.astype(bf), m2t.astype(bf)
    sel_np = np.zeros((4, 256), np.float32)
    for g in range(4):
        sel_np[g, g * 64:(g + 1) * 64] = 1.0
    sel_np = sel_np.astype(bf)
    in_maps = []
    for core in range(NCORES):
        b, c = core // 4, core % 4
        ks = 512 * c - 256
        xc = np.zeros((NK, DIM), np.float32)
        lo = max(0, ks)
        xc[lo - ks:] = x[b, lo: ks + NK]
        pos = ks + np.arange(NK, dtype=np.float32)
        freqs = pos[:, None] * inv_freq[None, :]        # [NK, 32]
        # cs[p, st*64 + j]: j<32 cos, j>=32 sin, for kv row st*128+p
        csk = np.concatenate([np.cos(freqs), np.sin(freqs)], axis=1)  # [NK, 64]
        csk = np.ascontiguousarray(
            csk.reshape(6, 128, HD).transpose(1, 0, 2).reshape(128, 6 * HD))
        kbias = np.where(pos < 0, -30000.0, 0.0).astype(np.float32)
        kbias = np.ascontiguousarray(kbias.reshape(6, 128).T)  # [128, 6]
        in_maps.append(
            {
                "xt": np.ascontiguousarray(xc.T).astype(bf),
                "wq": wqT,
                "wkv": wkvT,
                "wo": woT,
                "cs": csk,
                "kb": kbias,
                "qg8": qg8,
                "m0": m0t,
                "m2": m2t,
                "sel": sel_np,
            }
        )
    return in_maps


def kernel(x, Wq, Wk, Wv, Wo, q_gain, pair_mix):
    global _BUILT
    from concourse.bass_utils import run_bass_kernel_spmd

    if _BUILT is None:
        _BUILT = _build()
    in_maps = _host_inputs(x, Wq, Wk, Wv, Wo, q_gain, pair_mix)
    res = run_bass_kernel_spmd(_BUILT, in_maps, list(range(NCORES)))
    out = np.empty((B, S, DIM), np.float32)
    for core in range(NCORES):
        b, c = core // 4, core % 4
        out[b, 512 * c: 512 * c + 512, :] = res.results[core]["out"].astype(np.float32)
    return out


# revision 19
# speedup vs baseline: 1.1363x; 1.0781x over previous
"""Trainium2 Bass kernel for nn_BaselineGPT (sliding-window GQA attention block).

Sharding: 8 cores = 2 batches x 4 sequence chunks of 512 queries.
Each core computes its 512 output rows end-to-end (QKV proj, RMS norm, RoPE,
windowed GQA attention, output proj).  KV halo of 256 rows comes with the
chunk; chunk 0's missing halo is masked via a -30000 bias folded into the
exp() activation's per-partition bias slot.  K-side rmsnorm is folded into
the exp() scale slot (rope commutes with per-head scaling).  Pair-head
mixing is folded into Wo on the host.

Perf notes: the PE p-state ramp means the tensor engine runs 2x faster when
continuously busy, so instruction order keeps the tensor queue dense; DMA
loads are split across the sync/scalar/gpsimd queues in dependency order so
the first matmul can start ~10us in.
"""

import math
from contextlib import ExitStack

import numpy as np

import concourse.bass as bass
from concourse import bacc
import concourse.mybir as mybir
import concourse.tile as tile
from concourse.masks import make_identity

B, S, DIM = 2, 2048, 1024
H, KVH, HD = 16, 4, 64
WINDOW = 256
ROPE_BASE = 10000.0
EPS = 1e-6

NQ = 512          # queries per core
NK = 768          # kv rows per core (incl 256 halo)
NCORES = 8
F32 = mybir.dt.float32
BF16 = mybir.dt.bfloat16

_BUILT = None


def _build():
    nc = bacc.Bacc(None)

    xt = nc.declare_dram_parameter("xt", [DIM, NK], BF16, isOutput=False)
    wq = nc.declare_dram_parameter("wq", [DIM, DIM], BF16, isOutput=False)
    wkv = nc.declare_dram_parameter("wkv", [DIM, 512], BF16, isOutput=False)
    wo = nc.declare_dram_parameter("wo", [DIM, DIM], BF16, isOutput=False)
    cs = nc.declare_dram_parameter("cs", [128, 6 * HD], F32, isOutput=False)
    kb = nc.declare_dram_parameter("kb", [128, 6], F32, isOutput=False)
    qg8 = nc.declare_dram_parameter("qg8", [128, H], F32, isOutput=False)
    m0 = nc.declare_dram_parameter("m0", [128, 512], BF16, isOutput=False)
    m2 = nc.declare_dram_parameter("m2", [128, 512], BF16, isOutput=False)
    out = nc.declare_dram_parameter("out", [NQ, DIM], BF16, isOutput=True)

    with tile.TileContext(nc) as tc, ExitStack() as ctx:
        const = ctx.enter_context(tc.tile_pool(name="const", bufs=1))
        big = ctx.enter_context(tc.tile_pool(name="big", bufs=1))
        tmp = ctx.enter_context(tc.tile_pool(name="tmp", bufs=3))
        att_pool = ctx.enter_context(tc.tile_pool(name="att", bufs=5))
        ob_pool = ctx.enter_context(tc.tile_pool(name="ob", bufs=2))
        ps_proj = ctx.enter_context(tc.tile_pool(name="psp", bufs=3, space="PSUM"))
        ps_pss = ctx.enter_context(tc.tile_pool(name="pss", bufs=3, space="PSUM"))
        ps_y = ctx.enter_context(tc.tile_pool(name="psy", bufs=2, space="PSUM"))

        # ---- preload DMAs, ordered by first use across the 3 dma queues ----
        xt_sb = [None] * 8
        wkv_sb = [None] * 8
        wq_sb = [None] * 8
        wo_sb = [None] * 8
        for kt_ in range(8):
            xt_sb[kt_] = big.tile([128, NK], BF16, tag=f"xt{kt_}", name=f"xt{kt_}")
            wkv_sb[kt_] = big.tile([128, 512], BF16, tag=f"wkv{kt_}", name=f"wkv{kt_}")
            wq_sb[kt_] = big.tile([128, DIM], BF16, tag=f"wq{kt_}", name=f"wq{kt_}")
            wo_sb[kt_] = big.tile([128, DIM], BF16, tag=f"wo{kt_}", name=f"wo{kt_}")
        cs_sb = const.tile([128, 6, HD], F32, tag="cs")
        kb_sb = const.tile([128, 6], F32, tag="kb")
        qg_sb = const.tile([128, H], F32, tag="qg")
        m02_sb = const.tile([128, 2, 512], BF16, tag="m02")

        def dram_tile(t, dram, kt_):
            return (t, dram[kt_ * 128: kt_ * 128 + 128, :])

        # interleave so the kt=0..7 accumulation chain unblocks in order
        sync_q = [dram_tile(wkv_sb[0], wkv, 0), dram_tile(xt_sb[0], xt, 0),
                  dram_tile(xt_sb[1], xt, 1), dram_tile(xt_sb[2], xt, 2),
                  dram_tile(wkv_sb[1], wkv, 1), dram_tile(wkv_sb[2], wkv, 2),
                  dram_tile(wq_sb[0], wq, 0), dram_tile(wq_sb[1], wq, 1),
                  dram_tile(wq_sb[2], wq, 2), dram_tile(wq_sb[3], wq, 3),
                  (m02_sb[:, 0, :], m0[:, :]),
                  dram_tile(wo_sb[0], wo, 0), dram_tile(wo_sb[1], wo, 1),
                  dram_tile(wo_sb[2], wo, 2), dram_tile(wo_sb[3], wo, 3)]
        scal_q = [dram_tile(xt_sb[3], xt, 3), dram_tile(xt_sb[4], xt, 4),
                  dram_tile(xt_sb[5], xt, 5), dram_tile(wkv_sb[3], wkv, 3),
                  dram_tile(wkv_sb[4], wkv, 4),
                  dram_tile(wq_sb[4], wq, 4), dram_tile(wq_sb[5], wq, 5),
                  dram_tile(wq_sb[6], wq, 6), dram_tile(wq_sb[7], wq, 7),
                  (m02_sb[:, 1, :], m2[:, :]),
                  dram_tile(wo_sb[4], wo, 4), dram_tile(wo_sb[5], wo, 5),
                  dram_tile(wo_sb[6], wo, 6), dram_tile(wo_sb[7], wo, 7)]
        gps_q = [dram_tile(xt_sb[6], xt, 6), dram_tile(xt_sb[7], xt, 7),
                 (cs_sb.rearrange("p a b -> p (a b)"), cs[:, :]),
                 dram_tile(wkv_sb[5], wkv, 5), dram_tile(wkv_sb[6], wkv, 6),
                 dram_tile(wkv_sb[7], wkv, 7),
                 (kb_sb, kb[:, :]), (qg_sb, qg8[:, :])]
        for t, src in sync_q:
            nc.sync.dma_start(out=t, in_=src)
        for t, src in scal_q[:5]:
            nc.scalar.dma_start(out=t, in_=src)
        # preload activation tables (Exp/Sqrt/Copy) before first real use
        warm = const.tile([128, 1], F32, tag="warm")
        nc.vector.memset(warm, 1.0)
        nc.scalar.activation(out=warm, in_=warm,
                             func=mybir.ActivationFunctionType.Exp)
        nc.scalar.activation(out=warm, in_=warm,
                             func=mybir.ActivationFunctionType.Sqrt)
        nc.scalar.copy(out=warm, in_=warm)
        for t, src in scal_q[5:]:
            nc.scalar.dma_start(out=t, in_=src)
        for t, src in gps_q:
            nc.gpsimd.dma_start(out=t, in_=src)

        # ---- constants ----
        ident = const.tile([128, 128], BF16, tag="ident")
        make_identity(nc, ident)
        eps_t = const.tile([128, 1], F32, tag="eps")
        nc.vector.memset(eps_t, EPS)
        ones64 = const.tile([1, 64], BF16, tag="ones64")
        nc.vector.memset(ones64, 1.0)

        # ---- persistent SBUF tensors ----
        k_rope = big.tile([128, 6, KVH * HD], BF16, tag="krope")
        q_rope = big.tile([128, 4, DIM], BF16, tag="qrope")
        v_sb = big.tile([128, 6, KVH, HD + 1], BF16, tag="v")
        kt_sb = big.tile([64, KVH, NK], BF16, tag="kt")
        qt_sb = big.tile([64, 16, 512], BF16, tag="qt")
        yt_sb = big.tile([128, 8, 512], BF16, tag="yt")
        invk = big.tile([128, 6, KVH], F32, tag="invk")
        nc.vector.memset(v_sb[:, :, :, HD:HD + 1], 1.0)

        def rope(eng, dst, src, nh, st, tmp_tag):
            """dst[:, h, 0:32] = r1*cos + r2*sin ; dst[:, h, 32:64] = r2*cos - r1*sin"""
            hd2 = HD // 2
            r1 = src[:, :, 0:hd2]
            r2 = src[:, :, hd2:HD]
            cosb = cs_sb[:, st, 0:hd2].rearrange("p (o f) -> p o f", o=1).broadcast_to(
                (128, nh, hd2))
            sinb = cs_sb[:, st, hd2:HD].rearrange("p (o f) -> p o f", o=1).broadcast_to(
                (128, nh, hd2))
            t1 = tmp.tile([128, nh, hd2], BF16, tag=tmp_tag)
            t2 = tmp.tile([128, nh, hd2], BF16, tag=tmp_tag + "b")
            eng.tensor_mul(out=t1, in0=r1, in1=cosb)
            eng.tensor_mul(out=t2, in0=r2, in1=sinb)
            eng.tensor_add(out=dst[:, :, 0:hd2], in0=t1, in1=t2)
            eng.tensor_mul(out=t1, in0=r2, in1=cosb)
            eng.tensor_mul(out=t2, in0=r1, in1=sinb)
            eng.tensor_sub(out=dst[:, :, hd2:HD], in0=t1, in1=t2)

        # ---- emission units; finely interleaved so the in-order tensor
        # queue always has dep-ready work while other engines drain ----
        def emit_kv(st):
            pkv = ps_proj.tile([128, 512], F32, tag="pp")
            for kt_ in range(8):
                nc.tensor.matmul(
                    out=pkv,
                    lhsT=xt_sb[kt_][:, st * 128: st * 128 + 128],
                    rhs=wkv_sb[kt_],
                    start=(kt_ == 0),
                    stop=(kt_ == 7),
                )
            kraw = tmp.tile([128, KVH, HD], BF16, tag="kraw")
            nc.scalar.copy(
                out=kraw, in_=pkv[:, 0:KVH * HD].rearrange("p (g d) -> p g d", d=HD))
            kr = k_rope[:, st, :].rearrange("p (g d) -> p g d", d=HD)
            rope(nc.gpsimd, kr, kraw, KVH, st, "kr")
            nc.scalar.copy(
                out=v_sb[:, st, :, 0:HD],
                in_=pkv[:, KVH * HD:].rearrange("p (g d) -> p g d", d=HD),
            )
            sqk = tmp.tile([128, KVH, HD], F32, tag="sqk")
            nc.gpsimd.tensor_mul(out=sqk, in0=kr, in1=kr)
            ssqk = tmp.tile([128, KVH], F32, tag="ssqk")
            nc.vector.tensor_reduce(
                out=ssqk, in_=sqk,
                axis=mybir.AxisListType.X, op=mybir.AluOpType.add)
            nc.scalar.activation(
                out=ssqk, in_=ssqk, func=mybir.ActivationFunctionType.Sqrt,
                bias=eps_t, scale=1.0 / HD)
            nc.vector.reciprocal(out=invk[:, st, :], in_=ssqk)
            ptk = ps_pss.tile([128, 512], BF16, tag="ps")
            for g in range(KVH):
                nc.tensor.transpose(
                    out=ptk[0:HD, g * 128: g * 128 + 128],
                    in_=k_rope[:, st, g * HD: g * HD + HD],
                    identity=ident,
                )
            nc.vector.tensor_copy(
                out=kt_sb[:, :, st * 128: st * 128 + 128],
                in_=ptk[0:HD, :].rearrange("p (g s) -> p g s", s=128),
            )

        def emit_qhalf(st, half):
            qst = st - 2
            pq = ps_proj.tile([128, 512], F32, tag="pp")
            for kt_ in range(8):
                nc.tensor.matmul(
                    out=pq,
                    lhsT=xt_sb[kt_][:, st * 128: st * 128 + 128],
                    rhs=wq_sb[kt_][:, half * 512: half * 512 + 512],
                    start=(kt_ == 0),
                    stop=(kt_ == 7),
                )
            nc.scalar.copy(
                out=qraw_sb[:, half * 8: half * 8 + 8, :],
                in_=pq.rearrange("p (h d) -> p h d", d=HD),
            )
            if half == 1:
                qr = q_rope[:, qst, :].rearrange("p (h d) -> p h d", d=HD)
                rope(nc.vector, qr, qraw_sb, H, st, "qr")
                sqq = tmp.tile([128, H, HD], BF16, tag="sqq")
                ssqq = tmp.tile([128, H], F32, tag="ssqq")
                nc.gpsimd.tensor_mul(out=sqq, in0=qr, in1=qr)
                nc.vector.tensor_reduce(
                    out=ssqq, in_=sqq, axis=mybir.AxisListType.X,
                    op=mybir.AluOpType.add)
                nc.scalar.activation(
                    out=ssqq, in_=ssqq, func=mybir.ActivationFunctionType.Sqrt,
                    bias=eps_t, scale=1.0 / HD)
                invq = tmp.tile([128, H], F32, tag="invq")
                nc.vector.reciprocal(out=invq, in_=ssqq)
                nc.vector.tensor_mul(out=invq, in0=invq, in1=qg_sb)
                nc.vector.tensor_mul(
                    out=qr, in0=qr,
                    in1=invq.rearrange("p (h o) -> p h o", o=1).broadcast_to(
                        (128, H, HD)))

        def emit_qtrans(st):
            qst = st - 2
            for g in range(KVH):
                ptq = ps_pss.tile([128, 512], BF16, tag="ps")
                for hh in range(4):
                    h = g * 4 + hh
                    nc.tensor.transpose(
                        out=ptq[0:HD, hh * 128: hh * 128 + 128],
                        in_=q_rope[:, qst, h * HD: h * HD + HD],
                        identity=ident,
                    )
                nc.vector.tensor_copy(
                    out=qt_sb[:, g * 4 + qst, :], in_=ptq[0:HD, :])

        # att slots: t=0 -> 0, t=2 -> 1 (mask-adjacent), t=1 -> 2 (unmasked)
        SLOT = {0: 0, 2: 1, 1: 2}

        def emit_scores(qb, g):
            att = att_pool.tile([128, 3, 512], BF16, tag="att")
            for t in (0, 2, 1):
                pss = ps_pss.tile([128, 512], F32, tag="ps")
                nc.tensor.matmul(
                    out=pss,
                    lhsT=kt_sb[:, g, qb * 128 + t * 128: qb * 128 + t * 128 + 128],
                    rhs=qt_sb[:, g * 4 + qb, :],
                    start=True, stop=True,
                )
                nc.scalar.activation(
                    out=att[:, SLOT[t], :], in_=pss,
                    func=mybir.ActivationFunctionType.Exp,
                    bias=kb_sb[:, qb + t: qb + t + 1],
                    scale=invk[:, qb + t, g: g + 1],
                )
                if t == 2:
                    nc.vector.tensor_mul(
                        out=att[:, 0, :], in0=att[:, 0, :], in1=m02_sb[:, 0, :])
                    nc.gpsimd.tensor_mul(
                        out=att[:, 1, :], in0=att[:, 1, :], in1=m02_sb[:, 1, :])
            return att

        def emit_attv(qb, g, att):
            psy = ps_y.tile([128, 512], F32, tag="py")
            for t in (1, 0, 2):
                nc.tensor.matmul(
                    out=psy[0:HD + 1, :],
                    lhsT=v_sb[:, qb + t, g, :],
                    rhs=att[:, SLOT[t], :],
                    start=(t == 1), stop=(t == 2),
                )
            den_s = tmp.tile([1, 512], F32, tag="dens")
            nc.scalar.copy(out=den_s, in_=psy[64:65, :])
            rec = tmp.tile([1, 512], F32, tag="rec")
            nc.vector.reciprocal_approx_fast(out=rec, in_=den_s)
            rec_bf = tmp.tile([1, 512], BF16, tag="recb")
            nc.gpsimd.tensor_copy(out=rec_bf, in_=rec)
            return psy, rec_bf

        def emit_norm(qb, g, psy, rec_bf):
            prb = ps_pss.tile([128, 512], F32, tag="ps")
            nc.tensor.matmul(
                out=prb[0:HD, :], lhsT=ones64,
                rhs=rec_bf, start=True, stop=True)
            rbb = tmp.tile([HD, 512], BF16, tag="rbb")
            nc.vector.tensor_copy(out=rbb, in_=prb[0:HD, :])
            psy4 = psy[0:HD, :].rearrange("p (h s) -> p h s", s=128)
            prb4 = rbb.rearrange("p (h s) -> p h s", s=128)
            for lo in range(2):
                nc.vector.tensor_mul(
                    out=yt_sb[lo * 64: lo * 64 + 64, 2 * g: 2 * g + 2,
                              qb * 128: qb * 128 + 128],
                    in0=psy4[:, 2 * lo: 2 * lo + 2, :],
                    in1=prb4[:, 2 * lo: 2 * lo + 2, :],
                )

        def emit_outproj(qb, half):
            po = ps_pss.tile([128, 512], F32, tag="ps")
            for p in range(8):
                nc.tensor.matmul(
                    out=po,
                    lhsT=yt_sb[:, p, qb * 128: qb * 128 + 128],
                    rhs=wo_sb[p][:, half * 512: half * 512 + 512],
                    start=(p == 0), stop=(p == 7),
                )
            ob = ob_tiles[qb]
            nc.vector.tensor_copy(
                out=ob[:, half * 512: half * 512 + 512], in_=po)
            if half == 1:
                nc.sync.dma_start(out=out[qb * 128: qb * 128 + 128, :], in_=ob)

        qraw_sb = big.tile([128, H, HD], BF16, tag="qraw")
        ob_tiles = [big.tile([128, DIM], BF16, tag=f"ob{i}", name=f"ob{i}")
                    for i in range(4)]

        # schedule: SC/AV/OP units spaced by proj units so the tensor queue
        # never runs dry while scalar (exp) and vector (rope/norm) drain
        P, R = {}, {}

        def SC(qb, g):
            P.setdefault(qb, {})[g] = emit_scores(qb, g)

        def AV(qb, g):
            psy, rec = emit_attv(qb, g, P[qb][g])
            R.setdefault(qb, {})[g] = (psy, rec)
            if g >= 1:
                emit_norm(qb, g - 1, *R[qb][g - 1])
            if g == 3:
                emit_norm(qb, 3, *R[qb][3])

        emit_kv(0)
        emit_kv(1)
        emit_kv(2)
        emit_qhalf(2, 0); emit_qhalf(2, 1)
        emit_qtrans(2)
        SC(0, 0); SC(0, 1)
        emit_kv(3)
        SC(0, 2); SC(0, 3)
        emit_qhalf(3, 0)
        AV(0, 0)
        emit_qhalf(3, 1)
        AV(0, 1)
        emit_qtrans(3)
        AV(0, 2)
        emit_kv(4)
        AV(0, 3)
        SC(1, 0); SC(1, 1)
        emit_qhalf(4, 0)
        SC(1, 2); SC(1, 3)
        emit_qhalf(4, 1)
        AV(1, 0)
        emit_qtrans(4)
        AV(1, 1)
        emit_kv(5)
        AV(1, 2)
        emit_qhalf(5, 0)
        AV(1, 3)
        SC(2, 0); SC(2, 1)
        emit_qhalf(5, 1)
        SC(2, 2); SC(2, 3)
        emit_qtrans(5)
        AV(2, 0)
        emit_outproj(0, 0)
        AV(2, 1)
        emit_outproj(0, 1)
        AV(2, 2)
        emit_outproj(1, 0)
        AV(2, 3)
        SC(3, 0); SC(3, 1)
        emit_outproj(1, 1)
        SC(3, 2); SC(3, 3)
        AV(3, 0)
        emit_outproj(2, 0)
        AV(3, 1)
        emit_outproj(2, 1)
        AV(3, 2)
        AV(3, 3)
        emit_outproj(3, 0)
        emit_outproj(3, 1)

    nc.finalize()
    return nc


def _host_inputs(x, Wq, Wk, Wv, Wo, q_gain, pair_mix):
    """Build the 8 per-core input maps."""
    x = np.asarray(x, np.float32)
    Wq = np.asarray(Wq, np.float32)
    Wk = np.asarray(Wk, np.float32)
    Wv = np.asarray(Wv, np.float32)
    Wo = np.asarray(Wo, np.float32)
    q_gain = np.asarray(q_gain, np.float32)
    pair_mix = np.asarray(pair_mix, np.float32)

    # fold pair mixing into Wo:  out = y_mix @ Wo.T,  y_mix = y @ M.T  =>  Wo' = Wo @ M
    M = np.zeros((DIM, DIM), np.float32)
    eye = np.eye(HD, dtype=np.float32)
    for p in range(H // 2):
        for o in range(2):
            for i in range(2):
                ho, hi = 2 * p + o, 2 * p + i
                M[ho * HD: ho * HD + HD, hi * HD: hi * HD + HD] = (
                    pair_mix[p, o, i] * eye
                )
    woT = np.ascontiguousarray((Wo @ M).T)  # [in=(h,d), out]
    # permute rows into the yt pair layout: row blk*128 + lo*64 + d
    # holds head h = 4*(blk//2) + 2*lo + blk%2, dim d
    perm = np.empty(DIM, np.int64)
    for blk in range(8):
        for lo in range(2):
            h = 4 * (blk // 2) + 2 * lo + (blk % 2)
            perm[blk * 128 + lo * 64: blk * 128 + lo * 64 + HD] = (
                np.arange(HD) + h * HD)
    woT = woT[perm]

    wqT = np.ascontiguousarray(Wq.T)
    wkvT = np.ascontiguousarray(np.concatenate([Wk.T, Wv.T], axis=1))
    qg8 = np.tile((q_gain / math.sqrt(HD)).reshape(1, H), (128, 1)).astype(np.float32)

    inv_freq = 1.0 / (ROPE_BASE ** (np.arange(0, HD, 2, dtype=np.float32) / HD))

    ql = np.arange(128)
    m0_ = (ql[:, None] >= ql[None, :] + 1).astype(np.float32)  # kl >= ql+1
    m2_ = (ql[:, None] <= ql[None, :]).astype(np.float32)      # kl <= ql
    m0t = np.ascontiguousarray(np.tile(m0_, (1, 4)))
    m2t = np.ascontiguousarray(np.tile(m2_, (1, 4)))

    import ml_dtypes
    bf = ml_dtypes.bfloat16
    wqT, wkvT, woT = (a.astype(bf) for a in (wqT, wkvT, woT))
    m0t, m2t = m0t.astype(bf), m2t.astype(bf)
    sel_np = np.zeros((4, 256), np.float32)
    for g in range(4):
        sel_np[g, g * 64:(g + 1) * 64] = 1.0
    sel_np = sel_np.astype(bf)
    in_maps = []
    for core in range(NCORES):
        b, c = core // 4, core % 4
        ks = 512 * c - 256
        xc = np.zeros((NK, DIM), np.float32)
        lo = max(0, ks)
        xc[lo - ks:] = x[b, lo: ks + NK]
        pos = ks + np.arange(NK, dtype=np.float32)
        freqs = pos[:, None] * inv_freq[None, :]        # [NK, 32]
        # cs[p, st*64 + j]: j<32 cos, j>=32 sin, for kv row st*128+p
        csk = np.concatenate([np.cos(freqs), np.sin(freqs)], axis=1)  # [NK, 64]
        csk = np.ascontiguousarray(
            csk.reshape(6, 128, HD).transpose(1, 0, 2).reshape(128, 6 * HD))
        kbias = np.where(pos < 0, -30000.0, 0.0).astype(np.float32)
        kbias = np.ascontiguousarray(kbias.reshape(6, 128).T)  # [128, 6]
        in_maps.append(
            {
                "xt": np.ascontiguousarray(xc.T).astype(bf),
                "wq": wqT,
                "wkv": wkvT,
                "wo": woT,
                "cs": csk,
                "kb": kbias,
                "qg8": qg8,
                "m0": m0t,
                "m2": m2t,
                "sel": sel_np,
            }
        )
    return in_maps


def kernel(x, Wq, Wk, Wv, Wo, q_gain, pair_mix):
    global _BUILT
    from concourse.bass_utils import run_bass_kernel_spmd

    if _BUILT is None:
        _BUILT = _build()
    in_maps = _host_inputs(x, Wq, Wk, Wv, Wo, q_gain, pair_mix)
    res = run_bass_kernel_spmd(_BUILT, in_maps, list(range(NCORES)))
    out = np.empty((B, S, DIM), np.float32)
    for core in range(NCORES):
        b, c = core // 4, core % 4
        out[b, 512 * c: 512 * c + 512, :] = res.results[core]["out"].astype(np.float32)
    return out


# revision 30
# speedup vs baseline: 1.2367x; 1.0883x over previous
"""Trainium2 Bass kernel for nn_BaselineGPT (sliding-window GQA attention block).

Sharding: 8 cores = 2 batches x 4 sequence chunks of 512 queries.
Each core computes its 512 output rows end-to-end (QKV proj, RMS norm, RoPE,
windowed GQA attention, output proj).  KV halo of 256 rows comes with the
chunk; chunk 0's missing halo is masked via a -30000 bias folded into the
exp() activation's per-partition bias slot.  K-side rmsnorm is folded into
the exp() scale slot (rope commutes with per-head scaling).  Pair-head
mixing is folded into Wo on the host.

Perf notes: the PE p-state ramp means the tensor engine runs 2x faster when
continuously busy, so instruction order keeps the tensor queue dense; DMA
loads are split across the sync/scalar/gpsimd queues in dependency order so
the first matmul can start ~10us in.
"""

import math
from contextlib import ExitStack

import numpy as np

import concourse.bass as bass
from concourse import bacc
import concourse.mybir as mybir
import concourse.tile as tile
from concourse.masks import make_identity

B, S, DIM = 2, 2048, 1024
H, KVH, HD = 16, 4, 64
WINDOW = 256
ROPE_BASE = 10000.0
EPS = 1e-6

NQ = 512          # queries per core
NK = 768          # kv rows per core (incl 256 halo)
NCORES = 8
F32 = mybir.dt.float32
BF16 = mybir.dt.bfloat16

_BUILT = None


def _build():
    nc = bacc.Bacc(None)

    xt = nc.declare_dram_parameter("xt", [DIM, NK], BF16, isOutput=False)
    wq = nc.declare_dram_parameter("wq", [DIM, DIM], BF16, isOutput=False)
    wkv = nc.declare_dram_parameter("wkv", [DIM, 512], BF16, isOutput=False)
    wo = nc.declare_dram_parameter("wo", [DIM, DIM], BF16, isOutput=False)
    cs = nc.declare_dram_parameter("cs", [128, 6 * HD], F32, isOutput=False)
    kb = nc.declare_dram_parameter("kb", [128, 6], F32, isOutput=False)
    qg8 = nc.declare_dram_parameter("qg8", [128, H], F32, isOutput=False)
    m0 = nc.declare_dram_parameter("m0", [128, 512], BF16, isOutput=False)
    m2 = nc.declare_dram_parameter("m2", [128, 512], BF16, isOutput=False)
    out = nc.declare_dram_parameter("out", [NQ, DIM], BF16, isOutput=True)

    with tile.TileContext(nc) as tc, ExitStack() as ctx:
        const = ctx.enter_context(tc.tile_pool(name="const", bufs=1))
        big = ctx.enter_context(tc.tile_pool(name="big", bufs=1))
        tmp = ctx.enter_context(tc.tile_pool(name="tmp", bufs=3))
        att_pool = ctx.enter_context(tc.tile_pool(name="att", bufs=6))
        ob_pool = ctx.enter_context(tc.tile_pool(name="ob", bufs=2))
        ps_proj = ctx.enter_context(tc.tile_pool(name="psp", bufs=2, space="PSUM"))
        ps_pss = ctx.enter_context(tc.tile_pool(name="pss", bufs=3, space="PSUM"))
        ps_y = ctx.enter_context(tc.tile_pool(name="psy", bufs=3, space="PSUM"))

        # ---- preload DMAs, ordered by first use across the 3 dma queues ----
        xt_sb = [None] * 8
        wkv_sb = [None] * 8
        wq_sb = [None] * 8
        wo_sb = [None] * 8
        for kt_ in range(8):
            xt_sb[kt_] = big.tile([128, NK], BF16, tag=f"xt{kt_}", name=f"xt{kt_}")
            wkv_sb[kt_] = big.tile([128, 512], BF16, tag=f"wkv{kt_}", name=f"wkv{kt_}")
            wq_sb[kt_] = big.tile([128, DIM], BF16, tag=f"wq{kt_}", name=f"wq{kt_}")
            wo_sb[kt_] = big.tile([128, DIM], BF16, tag=f"wo{kt_}", name=f"wo{kt_}")
        cs_sb = const.tile([128, 6, HD], F32, tag="cs")
        kb_sb = const.tile([128, 6], F32, tag="kb")
        qg_sb = const.tile([128, H], F32, tag="qg")
        m02_sb = const.tile([128, 2, 512], BF16, tag="m02")

        def dram_tile(t, dram, kt_):
            return (t, dram[kt_ * 128: kt_ * 128 + 128, :])

        # interleave so the kt=0..7 accumulation chain unblocks in order
        sync_q = [dram_tile(wkv_sb[0], wkv, 0), dram_tile(xt_sb[0], xt, 0),
                  dram_tile(xt_sb[1], xt, 1), dram_tile(xt_sb[2], xt, 2),
                  dram_tile(wkv_sb[1], wkv, 1), dram_tile(wkv_sb[2], wkv, 2),
                  dram_tile(wq_sb[0], wq, 0), dram_tile(wq_sb[1], wq, 1),
                  dram_tile(wq_sb[2], wq, 2), dram_tile(wq_sb[3], wq, 3),
                  (m02_sb[:, 0, :], m0[:, :]),
                  dram_tile(wo_sb[0], wo, 0), dram_tile(wo_sb[1], wo, 1),
                  dram_tile(wo_sb[2], wo, 2), dram_tile(wo_sb[3], wo, 3)]
        scal_q = [dram_tile(xt_sb[3], xt, 3), dram_tile(xt_sb[4], xt, 4),
                  dram_tile(xt_sb[5], xt, 5), dram_tile(wkv_sb[3], wkv, 3),
                  dram_tile(wkv_sb[4], wkv, 4),
                  dram_tile(wq_sb[4], wq, 4), dram_tile(wq_sb[5], wq, 5),
                  dram_tile(wq_sb[6], wq, 6), dram_tile(wq_sb[7], wq, 7),
                  (m02_sb[:, 1, :], m2[:, :]),
                  dram_tile(wo_sb[4], wo, 4), dram_tile(wo_sb[5], wo, 5),
                  dram_tile(wo_sb[6], wo, 6), dram_tile(wo_sb[7], wo, 7)]
        gps_q = [dram_tile(xt_sb[6], xt, 6), dram_tile(xt_sb[7], xt, 7),
                 dram_tile(wkv_sb[5], wkv, 5), dram_tile(wkv_sb[6], wkv, 6),
                 dram_tile(wkv_sb[7], wkv, 7),
                 (cs_sb.rearrange("p a b -> p (a b)"), cs[:, :]),
                 (kb_sb, kb[:, :]), (qg_sb, qg8[:, :])]
        for t, src in sync_q:
            nc.sync.dma_start(out=t, in_=src)
        for t, src in scal_q[:5]:
            nc.scalar.dma_start(out=t, in_=src)
        # preload activation tables (Exp/Sqrt/Copy) before first real use
        warm = const.tile([128, 1], F32, tag="warm")
        nc.vector.memset(warm, 1.0)
        nc.scalar.activation(out=warm, in_=warm,
                             func=mybir.ActivationFunctionType.Exp)
        nc.scalar.activation(out=warm, in_=warm,
                             func=mybir.ActivationFunctionType.Sqrt)
        nc.scalar.copy(out=warm, in_=warm)
        for t, src in scal_q[5:]:
            nc.scalar.dma_start(out=t, in_=src)
        for t, src in gps_q:
            nc.gpsimd.dma_start(out=t, in_=src)

        # ---- constants ----
        ident = const.tile([128, 128], BF16, tag="ident")
        make_identity(nc, ident)
        eps_t = const.tile([128, 1], F32, tag="eps")
        nc.vector.memset(eps_t, EPS)
        ones64 = const.tile([1, 64], BF16, tag="ones64")
        nc.vector.memset(ones64, 1.0)
        kc_rsq = const.tile([128, 1], mybir.dt.int32, tag="kcrsq")
        nc.vector.memset(kc_rsq, 0x5F3759DF)
        c15 = const.tile([128, 1], F32, tag="c15")
        nc.vector.memset(c15, 1.5)

        # ---- persistent SBUF tensors ----
        k_rope = big.tile([128, 6, KVH * HD], BF16, tag="krope")
        q_rope = big.tile([128, 4, DIM], BF16, tag="qrope")
        v_sb = big.tile([128, 6, KVH, HD + 1], BF16, tag="v")
        kt_sb = big.tile([64, KVH, NK], BF16, tag="kt")
        qt_sb = big.tile([64, 16, 512], BF16, tag="qt")
        yt_sb = big.tile([128, 8, 512], BF16, tag="yt")
        invk = big.tile([128, 6, KVH], F32, tag="invk")
        nc.vector.memset(v_sb[:, :, :, HD:HD + 1], 1.0)

        I32 = mybir.dt.int32

        def rsqrt_newton(out, m, n, final_scale):
            """out = final_scale / sqrt(m), all on vector (no scalar sqrt ->
            no activation-table swap). 2 Newton iters, ~5e-6 rel err."""
            mc = tmp.tile([128, n], F32, tag="rsqm")
            nc.vector.tensor_scalar(
                out=mc, in0=m, scalar1=1e-6, scalar2=None,
                op0=mybir.AluOpType.max)
            m = mc
            t = tmp.tile([128, n], I32, tag="rsqt")
            nc.vector.tensor_scalar(
                out=t, in0=m.bitcast(I32), scalar1=1, scalar2=None,
                op0=mybir.AluOpType.logical_shift_right)
            yb = tmp.tile([128, n], I32, tag="rsqy")
            nc.vector.tensor_tensor(
                out=yb, in0=kc_rsq.broadcast_to((128, n)), in1=t,
                op=mybir.AluOpType.subtract)
            y = yb.bitcast(F32)
            h = tmp.tile([128, n], F32, tag="rsqh")
            u = tmp.tile([128, n], F32, tag="rsqu")
            for it in range(2):
                nc.vector.tensor_mul(out=h, in0=y, in1=y)
                nc.vector.tensor_mul(out=h, in0=h, in1=m)
                nc.vector.scalar_tensor_tensor(
                    out=u, in0=h, scalar=-0.5, in1=c15.broadcast_to((128, n)),
                    op0=mybir.AluOpType.mult, op1=mybir.AluOpType.add)
                if it == 0:
                    nc.vector.tensor_mul(out=y, in0=y, in1=u)
                else:
                    nc.vector.scalar_tensor_tensor(
                        out=out, in0=y, scalar=final_scale, in1=u,
                        op0=mybir.AluOpType.mult, op1=mybir.AluOpType.mult)

        def rope(eng, dst, src, nh, st, tmp_tag):
            """dst[:, h, 0:32] = r1*cos + r2*sin ; dst[:, h, 32:64] = r2*cos - r1*sin"""
            hd2 = HD // 2
            r1 = src[:, :, 0:hd2]
            r2 = src[:, :, hd2:HD]
            cosb = cs_sb[:, st, 0:hd2].rearrange("p (o f) -> p o f", o=1).broadcast_to(
                (128, nh, hd2))
            sinb = cs_sb[:, st, hd2:HD].rearrange("p (o f) -> p o f", o=1).broadcast_to(
                (128, nh, hd2))
            t1 = tmp.tile([128, nh, hd2], BF16, tag=tmp_tag)
            t2 = tmp.tile([128, nh, hd2], BF16, tag=tmp_tag + "b")
            eng.tensor_mul(out=t1, in0=r1, in1=cosb)
            eng.tensor_mul(out=t2, in0=r2, in1=sinb)
            eng.tensor_add(out=dst[:, :, 0:hd2], in0=t1, in1=t2)
            eng.tensor_mul(out=t1, in0=r2, in1=cosb)
            eng.tensor_mul(out=t2, in0=r1, in1=sinb)
            eng.tensor_sub(out=dst[:, :, hd2:HD], in0=t1, in1=t2)

        # ---- emission units; finely interleaved so the in-order tensor
        # queue always has dep-ready work while other engines drain ----
        def emit_kv(st, kraw2):
            pkv = ps_proj.tile([128, 512], F32, tag="pp")
            for kt_ in range(8):
                nc.tensor.matmul(
                    out=pkv,
                    lhsT=xt_sb[kt_][:, st * 128: st * 128 + 128],
                    rhs=wkv_sb[kt_],
                    start=(kt_ == 0),
                    stop=(kt_ == 7),
                )
            nc.scalar.copy(
                out=kraw2[:, st % 2, :, :],
                in_=pkv[:, 0:KVH * HD].rearrange("p (g d) -> p g d", d=HD))
            nc.scalar.copy(
                out=v_sb[:, st, :, 0:HD],
                in_=pkv[:, KVH * HD:].rearrange("p (g d) -> p g d", d=HD),
            )

        def emit_kpost(st0, kraw2):
            # rope + rms-inv + transpose for stiles st0, st0+1 in one batch
            hd2 = HD // 2
            kr2 = k_rope[:, st0:st0 + 2, :].rearrange("p s (g d) -> p s g d", d=HD)
            r1 = kraw2[:, :, :, 0:hd2]
            r2 = kraw2[:, :, :, hd2:HD]
            cosb = cs_sb[:, st0:st0 + 2, 0:hd2].rearrange(
                "p s (o f) -> p s o f", o=1).broadcast_to((128, 2, KVH, hd2))
            sinb = cs_sb[:, st0:st0 + 2, hd2:HD].rearrange(
                "p s (o f) -> p s o f", o=1).broadcast_to((128, 2, KVH, hd2))
            t1 = tmp.tile([128, 2, KVH, hd2], BF16, tag="krt1")
            t2 = tmp.tile([128, 2, KVH, hd2], BF16, tag="krt2")
            nc.gpsimd.tensor_mul(out=t1, in0=r1, in1=cosb)
            nc.gpsimd.tensor_mul(out=t2, in0=r2, in1=sinb)
            nc.gpsimd.tensor_add(out=kr2[:, :, :, 0:hd2], in0=t1, in1=t2)
            nc.gpsimd.tensor_mul(out=t1, in0=r2, in1=cosb)
            nc.gpsimd.tensor_mul(out=t2, in0=r1, in1=sinb)
            nc.gpsimd.tensor_sub(out=kr2[:, :, :, hd2:HD], in0=t1, in1=t2)
            sqk = tmp.tile([128, 2, KVH, HD], F32, tag="sqk")
            nc.gpsimd.tensor_mul(out=sqk, in0=kr2, in1=kr2)
            ssqk = tmp.tile([128, 2, KVH], F32, tag="ssqk")
            nc.vector.tensor_reduce(
                out=ssqk, in_=sqk,
                axis=mybir.AxisListType.X, op=mybir.AluOpType.add)
            rsqrt_newton(invk[:, st0:st0 + 2, :].rearrange("p s g -> p (s g)"),
                         ssqk.rearrange("p s g -> p (s g)"), 2 * KVH, 8.0)
            for st in (st0, st0 + 1):
                ptk = ps_pss.tile([128, 512], BF16, tag="ps")
                for g in range(KVH):
                    nc.tensor.transpose(
                        out=ptk[0:HD, g * 128: g * 128 + 128],
                        in_=k_rope[:, st, g * HD: g * HD + HD],
                        identity=ident,
                    )
                nc.vector.tensor_copy(
                    out=kt_sb[:, :, st * 128: st * 128 + 128],
                    in_=ptk[0:HD, :].rearrange("p (g s) -> p g s", s=128),
                )

        def emit_qhalf(st, half):
            qst = st - 2
            pq = ps_proj.tile([128, 512], F32, tag="pp")
            for kt_ in range(8):
                nc.tensor.matmul(
                    out=pq,
                    lhsT=xt_sb[kt_][:, st * 128: st * 128 + 128],
                    rhs=wq_sb[kt_][:, half * 512: half * 512 + 512],
                    start=(kt_ == 0),
                    stop=(kt_ == 7),
                )
            nc.scalar.copy(
                out=qraw_sb[:, half * 8: half * 8 + 8, :],
                in_=pq.rearrange("p (h d) -> p h d", d=HD),
            )
            if half == 1:
                qr = q_rope[:, qst, :].rearrange("p (h d) -> p h d", d=HD)
                rope(nc.vector, qr, qraw_sb, H, st, "qr")
                sqq = tmp.tile([128, H, HD], BF16, tag="sqq")
                ssqq = tmp.tile([128, H], F32, tag="ssqq")
                nc.gpsimd.tensor_mul(out=sqq, in0=qr, in1=qr)
                nc.vector.tensor_reduce(
                    out=ssqq, in_=sqq, axis=mybir.AxisListType.X,
                    op=mybir.AluOpType.add)
                invq = tmp.tile([128, H], F32, tag="invq")
                rsqrt_newton(invq, ssqq, H, 1.0)
                nc.vector.tensor_mul(out=invq, in0=invq, in1=qg_sb)
                nc.vector.tensor_mul(
                    out=qr, in0=qr,
                    in1=invq.rearrange("p (h o) -> p h o", o=1).broadcast_to(
                        (128, H, HD)))

        def emit_qtrans(st):
            qst = st - 2
            for g in range(KVH):
                ptq = ps_pss.tile([128, 512], BF16, tag="ps")
                for hh in range(4):
                    h = g * 4 + hh
                    nc.tensor.transpose(
                        out=ptq[0:HD, hh * 128: hh * 128 + 128],
                        in_=q_rope[:, qst, h * HD: h * HD + HD],
                        identity=ident,
                    )
                nc.vector.tensor_copy(
                    out=qt_sb[:, g * 4 + qst, :], in_=ptq[0:HD, :])

        # att slots: t=0 -> 0, t=2 -> 1 (mask-adjacent), t=1 -> 2 (unmasked)
        SLOT = {0: 0, 2: 1, 1: 2}

        def emit_scores(qb, g):
            att = att_pool.tile([128, 3, 512], BF16, tag="att")
            for t in (1, 0, 2):
                pss = ps_pss.tile([128, 512], F32, tag="ps")
                nc.tensor.matmul(
                    out=pss,
                    lhsT=kt_sb[:, g, qb * 128 + t * 128: qb * 128 + t * 128 + 128],
                    rhs=qt_sb[:, g * 4 + qb, :],
                    start=True, stop=True,
                )
                nc.scalar.activation(
                    out=att[:, SLOT[t], :], in_=pss,
                    func=mybir.ActivationFunctionType.Exp,
                    bias=kb_sb[:, qb + t: qb + t + 1],
                    scale=invk[:, qb + t, g: g + 1],
                )
                if t == 2:
                    nc.gpsimd.tensor_mul(
                        out=att[:, 0:2, :], in0=att[:, 0:2, :], in1=m02_sb)
            return att

        def emit_attv(qb, g, att):
            psy = ps_y.tile([128, 512], F32, tag="py")
            for t in (1, 0, 2):
                nc.tensor.matmul(
                    out=psy[0:HD + 1, :],
                    lhsT=v_sb[:, qb + t, g, :],
                    rhs=att[:, SLOT[t], :],
                    start=(t == 1), stop=(t == 2),
                )
            den_s = tmp.tile([1, 512], F32, tag="dens")
            nc.scalar.copy(out=den_s, in_=psy[64:65, :])
            rec = tmp.tile([1, 512], F32, tag="rec")
            nc.vector.reciprocal_approx_fast(out=rec, in_=den_s)
            rec_bf = tmp.tile([1, 512], BF16, tag="recb")
            nc.vector.tensor_copy(out=rec_bf, in_=rec)
            return psy, rec_bf

        def emit_norm(qb, g, psy, rec_bf):
            prb = ps_pss.tile([128, 512], F32, tag="ps")
            nc.tensor.matmul(
                out=prb[0:HD, :], lhsT=ones64,
                rhs=rec_bf, start=True, stop=True)
            rbb = tmp.tile([HD, 512], BF16, tag="rbb")
            nc.vector.tensor_copy(out=rbb, in_=prb[0:HD, :])
            psy4 = psy[0:HD, :].rearrange("p (h s) -> p h s", s=128)
            prb4 = rbb.rearrange("p (h s) -> p h s", s=128)
            for lo in range(2):
                nc.vector.tensor_mul(
                    out=yt_sb[lo * 64: lo * 64 + 64, 2 * g: 2 * g + 2,
                              qb * 128: qb * 128 + 128],
                    in0=psy4[:, 2 * lo: 2 * lo + 2, :],
                    in1=prb4[:, 2 * lo: 2 * lo + 2, :],
                )

        def emit_outproj(qb, half):
            po = ps_pss.tile([128, 512], F32, tag="ps")
            for p in range(8):
                nc.tensor.matmul(
                    out=po,
                    lhsT=yt_sb[:, p, qb * 128: qb * 128 + 128],
                    rhs=wo_sb[p][:, half * 512: half * 512 + 512],
                    start=(p == 0), stop=(p == 7),
                )
            ob = ob_tiles[qb]
            nc.vector.tensor_copy(
                out=ob[:, half * 512: half * 512 + 512], in_=po)
            if half == 1:
                nc.sync.dma_start(out=out[qb * 128: qb * 128 + 128, :], in_=ob)

        qraw_sb = big.tile([128, H, HD], BF16, tag="qraw")
        ob_tiles = [big.tile([128, DIM], BF16, tag=f"ob{i}", name=f"ob{i}")
                    for i in range(4)]

        # schedule: SC/AV/OP units spaced by proj units so the tensor queue
        # never runs dry while scalar (exp) and vector (rope/norm) drain
        P, R = {}, {}

        def SC(qb, g):
            P.setdefault(qb, {})[g] = emit_scores(qb, g)

        def AV(qb, g):
            psy, rec = emit_attv(qb, g, P[qb][g])
            R.setdefault(qb, {})[g] = (psy, rec)
            if g >= 1:
                emit_norm(qb, g - 1, *R[qb][g - 1])
            if g == 3:
                emit_norm(qb, 3, *R[qb][3])

        kraws = {}
        for pr_ in (0, 2, 4):
            kraws[pr_] = tmp.tile([128, 2, KVH, HD], BF16, tag=f"kraw{pr_}",
                                  name=f"kraw{pr_}")
        emit_kv(0, kraws[0])
        emit_kv(1, kraws[0])
        emit_kpost(0, kraws[0])
        emit_kv(2, kraws[2])
        emit_kv(3, kraws[2])
        emit_kpost(2, kraws[2])
        emit_qhalf(2, 0); emit_qhalf(2, 1)
        emit_qtrans(2)
        SC(0, 0); SC(0, 1)
        emit_kv(4, kraws[4])
        SC(0, 2); SC(0, 3)
        emit_qhalf(3, 0)
        AV(0, 0)
        emit_qhalf(3, 1)
        AV(0, 1)
        emit_qtrans(3)
        AV(0, 2)
        emit_kv(5, kraws[4])
        AV(0, 3)
        emit_kpost(4, kraws[4])
        SC(1, 0); SC(1, 1)
        emit_qhalf(4, 0)
        SC(1, 2); SC(1, 3)
        emit_qhalf(4, 1)
        AV(1, 0)
        emit_qtrans(4)
        AV(1, 1)
        AV(1, 2)
        emit_qhalf(5, 0)
        AV(1, 3)
        SC(2, 0); SC(2, 1)
        emit_qhalf(5, 1)
        SC(2, 2); SC(2, 3)
        emit_qtrans(5)
        AV(2, 0)
        emit_outproj(0, 0)
        AV(2, 1)
        emit_outproj(0, 1)
        AV(2, 2)
        emit_outproj(1, 0)
        AV(2, 3)
        SC(3, 0); SC(3, 1)
        emit_outproj(1, 1)
        SC(3, 2); SC(3, 3)
        AV(3, 0)
        emit_outproj(2, 0)
        AV(3, 1)
        emit_outproj(2, 1)
        AV(3, 2)
        AV(3, 3)
        emit_outproj(3, 0)
        emit_outproj(3, 1)